# revision 31
# baseline (speedup 1.0000x reference)
"""Multi-head attention (softmax over the HEADS axis) on 8 trn2 NeuronCores.

Reference math (B=2, S=2048, D=512, H=8, Dk=64):
    q = split_heads(Q @ w_q.T + b_q)          # [B,H,S,Dk]
    scores = q @ k.T / sqrt(Dk)               # [B,H,Sq,Sk]
    probs = softmax(scores, axis=1)           # softmax over H (source quirk!)
    attn = probs @ v                          # [B,H,Sq,Dk]
    out = concat_heads(attn) @ w_o.T + b_o    # [B,S,D]

Softmax over H is local to each (b, sq, sk) position: sharding over
(batch x query rows) needs no cross-core communication.  Core c handles
batch c//4, query rows (c%4)*512 .. +512, with all 8 heads resident.

Design notes (learned from perfetto traces):
 - Input DMA is HBM-bound (~21 us for 7.5 MB/core) and rings share
   bandwidth proportionally to DESCRIPTOR SIZE, so kt/vt ship in a
   kb-major DRAM layout where every transfer moves 4KB-contiguous rows.
   Stages are striped over the three rings (sync/scalar HWDGE + gpsimd
   SWDGE) in dependency order: qt+wqt first, then wkt+K blocks, then
   wvt+V blocks, weights for the output projection last.
 - Steady-state loop is ACT-bound (4 EXPs/iter) with DVE a close second
   (head-sum tree + recip + normalize mul).  K blocks 2-3 and all V-tile
   projections are emitted inside the loop to shorten the serial
   prologue; V tiles project in pairs so the psum->SBUF copy (on ACT)
   costs one instruction per two tiles.
 - Normalize MULs for iterations 12-14 run on GPSIMD (pure SBUF op,
   3-iteration pipeline slack absorbs its erratic latency) so the
   end-of-loop DVE FIFO drains fast and the tail's critical chain is
   only tree(15) -> recip(15) -> mul(15a) before the last PV tile.
 - Tail: PV tiles 12+13 drain inside iteration 15; 14 right after; 15
   interleaves per head-pair with attnT copies and the c-outer output
   projection (psum accumulation order over chunks is free).  Output is
   written bf16 (error budget allows) and DMAd per qi-block.
 - Dummy matmuls bridge every PE-idle window (DMA startup, drain waits)
   to keep the HAM clock-gate at 8/8.
"""

import numpy as np

B, S, D, H, DK = 2, 2048, 512, 8, 64
NCORES = 8
CPB = NCORES // B          # cores per batch
QI = S // CPB              # query rows per core (512)
KJT = 128                  # kj tile (partition dim of scores)
NKJ = S // KJT             # 16 kj tiles
NC_, CH = 128, 4           # partitions, din chunks
KB = 4                     # 512-column blocks of kj
QS = S // KB               # 512
SCALE = 1.0 / np.sqrt(DK)  # folded into exp activation
LAG = 3                    # PV matmuls run LAG kj tiles behind the softmax


def _chunk(x, dt):
    """[512, F] -> [128, 4, F] with row = chunk*128 + p."""
    f = x.shape[1]
    return np.ascontiguousarray(
        np.ascontiguousarray(x).reshape(CH, NC_, f).transpose(1, 0, 2)
    ).astype(dt)


def _chunk_kb(x, dt):
    """[512, 2048] -> [128, kb, c, 512], row = c*128+p, col = kb*512+j.

    kb-major so each kb block is one DMA with 4KB-contiguous descriptors.
    """
    y = _chunk(x, np.float32)                      # [128, c, 2048]
    y = y.reshape(NC_, CH, KB, QS).transpose(0, 2, 1, 3)  # [128, kb, c, 512]
    return np.ascontiguousarray(y).astype(dt)


def _build(with_bias):
    from contextlib import ExitStack

    import concourse.bass as bass
    import concourse.mybir as mybir
    import concourse.tile as tile
    from concourse import bacc
    from concourse.dve_ops import (
        RECIP_APPROX_FAST_CONSTS as _RC,
        RECIPROCAL_APPROX_FAST as _RF,
    )

    fp32 = mybir.dt.float32
    bf16 = mybir.dt.bfloat16

    nc = bacc.Bacc(
        "TRN2",
        target_bir_lowering=False,
        debug=False,
        enable_asserts=False,
        num_devices=NCORES,
    )

    def din(name, shape):
        return nc.dram_tensor(name, shape, bf16, kind="ExternalInput").ap()

    qt_d = din("qt", [NC_, CH, QI])
    kt_d = din("kt", [NC_, KB, CH, QS])
    vt_d = din("vt", [NC_, KB, CH, QS])
    w_d = {n: din(n, [NC_, CH, D]) for n in ("wqt", "wkt", "wvt", "wot")}
    if with_bias:
        b_d = {n: din(n, [1, D]) for n in ("bq", "bk", "bv", "bo")}
    out_d = nc.dram_tensor("out", [QI, D], bf16, kind="ExternalOutput").ap()

    with tile.TileContext(nc) as tc, ExitStack() as ctx:
        acts = ctx.enter_context(tc.tile_pool(name="acts", bufs=1))
        sm = ctx.enter_context(tc.tile_pool(name="sm", bufs=2))
        pp = ctx.enter_context(tc.tile_pool(name="pp", bufs=2))
        ps = ctx.enter_context(tc.tile_pool(name="ps", bufs=2, space="PSUM"))
        psa = ctx.enter_context(tc.tile_pool(name="psa", bufs=1, space="PSUM"))

        # ---- persistent SBUF tiles ----
        qTs = acts.tile([NC_, CH, QI], bf16, tag="qTs")
        kTsb = [
            acts.tile([NC_, CH, 1024], bf16, tag=f"kTs{b_}", name=f"kTs{b_}")
            for b_ in range(2)
        ]
        vs = acts.tile([NC_, NKJ, D], bf16, tag="vs", name="vs")
        attnT = acts.tile([NC_, CH, QI], bf16, tag="attnT")
        outsb = acts.tile([NC_, CH, D], bf16, tag="outsb")
        qraw = acts.tile([NC_, CH, QI], bf16, tag="qraw")
        kraw = acts.tile([NC_, KB, CH, QS], bf16, tag="kraw", name="kraw")
        vraw = acts.tile([NC_, KB, CH, QS], bf16, tag="vraw", name="vraw")
        wsb = {
            n: acts.tile([NC_, CH, D], bf16, tag=n, name=n)
            for n in ("wqt", "wkt", "wvt", "wot")
        }

        if with_bias:
            ones = acts.tile([1, 2 * D], bf16, tag="ones")
            nc.vector.memset(ones, 1.0)
            brow = {}
            for n in ("bq", "bk", "bv", "bo"):
                brow[n] = acts.tile([1, D], bf16, tag=n, name=n)
                nc.sync.dma_start(out=brow[n], in_=b_d[n])

        # ---- prologue DMAs: dependency-priority stages, 4KB descriptors ---
        # S1: Q proj inputs (+ wkt)
        nc.sync.dma_start(out=qraw, in_=qt_d)
        nc.scalar.dma_start(out=wsb["wqt"], in_=w_d["wqt"])
        nc.gpsimd.dma_start(out=wsb["wkt"], in_=w_d["wkt"])
        # S2: K blocks 0-1 (+ wvt)
        nc.sync.dma_start(out=kraw[:, 0], in_=kt_d[:, 0])
        nc.scalar.dma_start(out=kraw[:, 1], in_=kt_d[:, 1])
        nc.gpsimd.dma_start(out=wsb["wvt"], in_=w_d["wvt"])
        # S3: K blocks 2-3, V block 0
        nc.sync.dma_start(out=kraw[:, 2], in_=kt_d[:, 2])
        nc.scalar.dma_start(out=kraw[:, 3], in_=kt_d[:, 3])
        nc.gpsimd.dma_start(out=vraw[:, 0], in_=vt_d[:, 0])
        # S4: V blocks 1-2 (+ wot)
        nc.sync.dma_start(out=vraw[:, 1], in_=vt_d[:, 1])
        nc.scalar.dma_start(out=vraw[:, 2], in_=vt_d[:, 2])
        nc.gpsimd.dma_start(out=wsb["wot"], in_=w_d["wot"])
        # S5: V block 3
        nc.sync.dma_start(out=vraw[:, 3], in_=vt_d[:, 3])

        # dummy matmuls on garbage SBUF bridge PE-idle windows (DMA startup,
        # drain waits) so the HAM clock gate stays at 8/8.  Each call takes a
        # fresh psum tile from the rotating pool so it can never alias a
        # live score tile.
        _dn = [0]

        def dummy_mm(k):
            _dn[0] += 1
            wt = ps.tile([NC_, 2, 512], fp32, tag="sc", name=f"warm{_dn[0]}")
            for i in range(k):
                nc.tensor.matmul(
                    wt[:, i % 2, :],
                    lhsT=qTs[:, 0, 0:128],
                    rhs=qTs[:, 0, :],
                    start=True,
                    stop=True,
                )

        dummy_mm(12)

        def bias_mm(pt_ap, bname, col_slice):
            """rank-1 bias init: psum = bias-row (x) ones-row (or flipped)."""
            if col_slice is not None:  # bias along partitions
                lhsT = brow[bname][:, col_slice]
                rhs = ones[:, : pt_ap.shape[-1]]
            else:  # bias along free dim
                lhsT = ones[:, :128]
                rhs = brow[bname]
            nc.tensor.matmul(pt_ap, lhsT=lhsT, rhs=rhs, start=True, stop=False)

        # ---------------- prologue projections ----------------
        # Q: qT[dout, qi] = wqT[din,dout].T @ QT[din, qi]; two m per psum tile
        for mp in range(2):
            pt = ps.tile([NC_, 2, 512], fp32, tag="sc", name=f"qp{mp}")
            for j in range(2):
                m = 2 * mp + j
                if with_bias:
                    bias_mm(pt[:, j, :QI], "bq", slice(m * 128, (m + 1) * 128))
                for c in range(CH):
                    nc.tensor.matmul(
                        pt[:, j, :QI],
                        lhsT=wsb["wqt"][:, c, m * 128 : (m + 1) * 128],
                        rhs=qraw[:, c, :],
                        start=(c == 0 and not with_bias),
                        stop=(c == CH - 1),
                    )
            if mp == 0:
                nc.scalar.copy(qTs[:, 0:2, :], pt[:, :, :QI])
            else:
                nc.vector.tensor_copy(qTs[:, 2:4, :], pt[:, :, :QI])

        def emit_kproj(kb, mp):
            """one dout m-pair of K proj for 512-col block kb + kTs copy."""
            pt = ps.tile([NC_, 2, 512], fp32, tag="sc", name=f"kp{kb}_{mp}")
            for j in range(2):
                m = 2 * mp + j
                if with_bias:
                    bias_mm(pt[:, j, :], "bk", slice(m * 128, (m + 1) * 128))
                for c in range(CH):
                    nc.tensor.matmul(
                        pt[:, j, :],
                        lhsT=wsb["wkt"][:, c, m * 128 : (m + 1) * 128],
                        rhs=kraw[:, kb, c, :],
                        start=(c == 0 and not with_bias),
                        stop=(c == CH - 1),
                    )
            dst = kTsb[kb // 2][
                :, 2 * mp : 2 * mp + 2, (kb % 2) * 512 : (kb % 2 + 1) * 512
            ]
            if mp == 0:
                nc.scalar.copy(dst, pt)
            else:
                nc.vector.tensor_copy(dst, pt)

        # bridge the kb0 DMA-arrival gap after Q proj with dummies
        dummy_mm(4)
        # All K blocks in prologue.  (In-loop projection blocks serialize on
        # the 2-slot psum rotation and cost ~2x their naive service time --
        # measured three times.)
        for kb in range(KB):
            for mp in range(2):
                emit_kproj(kb, mp)

        def emit_vpair(t0, engine):
            """project v tiles t0, t0+1 into one psum tile + one copy."""
            kvt = ps.tile([NC_, 2, 512], fp32, tag="sc", name=f"vp{t0}")
            for j in range(2):
                vt_i = t0 + j
                if with_bias:
                    bias_mm(kvt[:, j, :], "bv", None)
                for c in range(CH):
                    nc.tensor.matmul(
                        kvt[:, j, :],
                        lhsT=vraw[
                            :, vt_i // 4, c, (vt_i % 4) * 128 : (vt_i % 4 + 1) * 128
                        ],
                        rhs=wsb["wvt"][:, c, :],
                        start=(c == 0 and not with_bias),
                        stop=(c == CH - 1),
                    )
            engine(vs[:, t0 : t0 + 2, :], kvt)

        emit_vpair(0, nc.vector.tensor_copy)
        emit_vpair(2, nc.scalar.copy)

        # ---------------- fused attention + pipelined K/V projection ------
        at = [
            psa.tile([NC_, 512], fp32, tag=f"at{i}", name=f"at{i}")
            for i in range(4)
        ]

        def emit_pv(td, pr, m):
            """PV matmuls for heads 2m, 2m+1 of kj tile td."""
            for h in (2 * m, 2 * m + 1):
                po = (h % 2) * 64
                nc.tensor.matmul(
                    at[h // 2][po : po + 64, :QI],
                    lhsT=vs[:, td, h * 64 : (h + 1) * 64],
                    rhs=pr[:, h, :],
                    start=(td == 0),
                    stop=(td == NKJ - 1),
                )

        # Softmax chains are batched over iteration PAIRS (2k, 2k+1) for
        # t<14: one s4/s2/ssum/recip/mul covering both iterations halves the
        # DVE instruction overhead (~0.5 us/iter) and shrinks the DVE FIFO
        # backlog that otherwise drains serially after the last EXP.
        # Iterations 14/15 share the last pair tile but run SOLO chains so
        # pr14/pr15 are available as early as possible for the tail.
        pending = []
        exp2 = None
        for t in range(NKJ):
            if t % 2 == 0:
                exp2 = sm.tile(
                    [NC_, 2, H, QI], bf16, tag="exp2", bufs=2, name=f"e{t}"
                )
            ei = t % 2
            npop = 0 if t < 4 else (2 if t >= NKJ - 2 else 1)
            pvs = [pending.pop(0) for _ in range(npop)]
            do_kv = t < NKJ - CH
            last = t == NKJ - 1
            if last:
                sA = sm.tile([NC_, 2, QI], bf16, tag="sA", bufs=1)

            for m in range(4):
                spt = ps.tile([NC_, 2, 512], fp32, tag="sc", name=f"s{t}_{m}")
                for j in range(2):
                    po = j * 64
                    nc.tensor.matmul(
                        spt[:, j, :QI],
                        lhsT=kTsb[t // 8][
                            po : po + 64, m, (t % 8) * 128 : (t % 8 + 1) * 128
                        ],
                        rhs=qTs[po : po + 64, m, :],
                        start=True,
                        stop=True,
                    )
                for pvt in pvs:
                    emit_pv(pvt[0], pvt[1], m)
                nc.scalar.activation(
                    exp2[:, ei, 2 * m : 2 * m + 2, :],
                    spt[:, :, :],
                    mybir.ActivationFunctionType.Exp,
                    scale=SCALE,
                )
                if m == 1:
                    if last:
                        # latency-optimized tree for the final tile
                        nc.vector.tensor_add(
                            sA, exp2[:, ei, 0:2, :], exp2[:, ei, 2:4, :]
                        )
                    if t < 4:
                        # pre-PV iterations are PE-sparse: cheap warm-keeper
                        # (a dummy slot releases after its own MMs, ~0.4 us,
                        # unlike projection slots which wait on ACT copies)
                        dummy_mm(2)
                    if do_kv:
                        # single V tile per iteration: a [128,1,512] psum
                        # tile keeps the 2-slot rotation fluid
                        vt_i = t + CH
                        kvt = ps.tile(
                            [NC_, 1, 512], fp32, tag="sc", name=f"kv{t}"
                        )
                        if with_bias:
                            bias_mm(kvt[:, 0, :], "bv", None)
                        for c in range(CH):
                            nc.tensor.matmul(
                                kvt[:, 0, :],
                                lhsT=vraw[
                                    :,
                                    vt_i // 4,
                                    c,
                                    (vt_i % 4) * 128 : (vt_i % 4 + 1) * 128,
                                ],
                                rhs=wsb["wvt"][:, c, :],
                                start=(c == 0 and not with_bias),
                                stop=(c == CH - 1),
                            )
                        nc.scalar.copy(vs[:, vt_i, :], kvt[:, 0, :])

            if t < NKJ - 2:
                if t % 2 == 0:
                    continue  # chain runs once per pair, after the odd iter
                # paired head-sum tree + recip + normalize on DVE
                s4p = sm.tile([NC_, 2, 4, QI], bf16, tag="s4p", bufs=1)
                nc.vector.tensor_add(
                    s4p, exp2[:, :, 0:4, :], exp2[:, :, 4:8, :]
                )
                s2p = sm.tile([NC_, 2, 2, QI], bf16, tag="s2p", bufs=1)
                nc.vector.tensor_add(s2p, s4p[:, :, 0:2, :], s4p[:, :, 2:4, :])
                ssump = sm.tile([NC_, 2, QI], bf16, tag="ssump", bufs=1)
                nc.vector.tensor_add(ssump, s2p[:, :, 0, :], s2p[:, :, 1, :])
                rp = sm.tile([NC_, 2, QI], bf16, tag="rp", bufs=1)
                nc.vector._custom_dve(
                    _RF,
                    out=rp,
                    in0=ssump,
                    s0=_RC["s0"],
                    s1=_RC["s1"],
                    imm2=_RC["imm2"],
                )
                pr2 = pp.tile(
                    [NC_, 2, H, QI], bf16, tag="probs2", bufs=2, name=f"pr{t}"
                )
                nc.vector.tensor_mul(
                    pr2,
                    exp2,
                    rp[:, :, None, :].broadcast_to([NC_, 2, H, QI]),
                )
                pending.append((t - 1, pr2[:, 0]))
                pending.append((t, pr2[:, 1]))
                continue

            # solo chains for iterations 14 and 15
            ssum = sm.tile([NC_, QI], bf16, tag="ssum", bufs=1)
            if last:
                sB = sm.tile([NC_, 2, QI], bf16, tag="sB", bufs=1)
                nc.vector.tensor_add(
                    sB, exp2[:, ei, 4:6, :], exp2[:, ei, 6:8, :]
                )
                sAB = sm.tile([NC_, 2, QI], bf16, tag="s2", bufs=1)
                nc.vector.tensor_add(sAB, sA, sB)
                nc.vector.tensor_add(ssum, sAB[:, 0, :], sAB[:, 1, :])
            else:
                s4 = sm.tile([NC_, 4, QI], bf16, tag="s4", bufs=1)
                nc.vector.tensor_add(
                    s4, exp2[:, ei, 0:4, :], exp2[:, ei, 4:8, :]
                )
                s2 = sm.tile([NC_, 2, QI], bf16, tag="s2", bufs=1)
                nc.vector.tensor_add(s2, s4[:, 0:2, :], s4[:, 2:4, :])
                nc.vector.tensor_add(ssum, s2[:, 0, :], s2[:, 1, :])
            r = sm.tile([NC_, QI], bf16, tag="r", bufs=2)
            nc.vector._custom_dve(
                _RF, out=r, in0=ssum, s0=_RC["s0"], s1=_RC["s1"], imm2=_RC["imm2"]
            )
            # split the mul so the tail's PV matmuls start after half
            pr = pp.tile([NC_, H, QI], bf16, tag="probs1", name=f"pr{t}")
            nc.vector.tensor_mul(
                pr[:, 0:4, :],
                exp2[:, ei, 0:4, :],
                r[:, None, :].broadcast_to([NC_, 4, QI]),
            )
            nc.vector.tensor_mul(
                pr[:, 4:8, :],
                exp2[:, ei, 4:8, :],
                r[:, None, :].broadcast_to([NC_, 4, QI]),
            )
            pending.append((t, pr))

        # ---------------- tail: drain + output projection, interleaved ----
        # Warm-keeper dummies fill the PE-idle waits on the softmax chain so
        # the HAM clock gate stays at 8/8 through the drain (measured: the
        # whole tail ran at 1.2 GHz without them).  One psum tile serves all
        # pre-otq bursts; later bursts write the already-copied (dead)
        # attn banks.
        (td14, pr14), (td15, pr15) = pending
        wt = ps.tile([NC_, 2, 512], fp32, tag="sc", name="warmtail")

        def dummy_wt(k):
            for i in range(k):
                nc.tensor.matmul(
                    wt[:, i % 2, :],
                    lhsT=qTs[:, 0, 0:128],
                    rhs=qTs[:, 0, :],
                    start=True,
                    stop=True,
                )

        dummy_wt(4)
        for m in range(4):
            emit_pv(td14, pr14, m)
            dummy_wt(2)

        otq = [
            ps.tile([NC_, 2, 512], fp32, tag="sc", name=f"oq{i}")
            for i in range(2)
        ]
        if with_bias:
            for qb in range(4):
                bias_mm(otq[qb // 2][:, qb % 2, :], "bo", None)
        for m in range(4):
            emit_pv(td15, pr15, m)
            if m % 2 == 0:
                nc.vector.tensor_copy(attnT[:, m, :], at[m][:, :QI])
            else:
                nc.scalar.copy(attnT[:, m, :], at[m][:, :QI])
            for qb in range(4):
                nc.tensor.matmul(
                    otq[qb // 2][:, qb % 2, :],
                    lhsT=attnT[:, m, qb * 128 : (qb + 1) * 128],
                    rhs=wsb["wot"][:, m, :],
                    start=(m == 0 and not with_bias),
                    stop=(m == CH - 1),
                )

        out_re = out_d.rearrange("(m p) o -> p m o", p=NC_)
        for qb in range(4):
            if qb % 2 == 0:
                nc.scalar.copy(outsb[:, qb, :], otq[qb // 2][:, qb % 2, :])
            else:
                nc.vector.tensor_copy(outsb[:, qb, :], otq[qb // 2][:, qb % 2, :])
            (nc.sync if qb % 2 == 0 else nc.scalar).dma_start(
                out=out_re[:, qb, :], in_=outsb[:, qb, :]
            )

    nc.compile()
    return nc


_CACHE = {}


def kernel(Q, K, V, w_q, b_q, w_k, b_k, w_v, b_v, w_o, b_o, _trace=False):
    import ml_dtypes
    from concourse import bass_utils

    bf = ml_dtypes.bfloat16
    Q = np.asarray(Q, np.float32)
    K = np.asarray(K, np.float32)
    V = np.asarray(V, np.float32)
    with_bias = any(
        np.any(np.asarray(b) != 0) for b in (b_q, b_k, b_v, b_o)
    )

    if ("nc", with_bias) not in _CACHE:
        _CACHE[("nc", with_bias)] = _build(with_bias)
    nc = _CACHE[("nc", with_bias)]

    wmaps = {
        "wqt": _chunk(np.asarray(w_q, np.float32).T, bf),
        "wkt": _chunk(np.asarray(w_k, np.float32).T, bf),
        "wvt": _chunk(np.asarray(w_v, np.float32).T, bf),
        "wot": _chunk(np.asarray(w_o, np.float32).T, bf),
    }
    if with_bias:
        for n, b in (("bq", b_q), ("bk", b_k), ("bv", b_v), ("bo", b_o)):
            wmaps[n] = np.ascontiguousarray(
                np.asarray(b, np.float32).reshape(1, D)
            ).astype(bf)

    in_maps = []
    for c in range(NCORES):
        b = c // CPB
        s0 = (c % CPB) * QI
        in_maps.append(
            dict(
                wmaps,
                qt=_chunk(Q[b, s0 : s0 + QI, :].T, bf),
                kt=_chunk_kb(K[b].T, bf),
                vt=_chunk_kb(V[b].T, bf),
            )
        )

    res = bass_utils.run_bass_kernel_spmd(
        nc, in_maps, core_ids=list(range(NCORES)), trace=_trace
    )

    out = np.empty((B, S, D), np.float32)
    for c in range(NCORES):
        b = c // CPB
        s0 = (c % CPB) * QI
        out[b, s0 : s0 + QI, :] = np.asarray(res.results[c]["out"]).astype(
            np.float32
        )
    if _trace:
        kernel._last_results = res
    return out


# revision 34
# speedup vs baseline: 1.2195x; 1.2195x over previous
"""Multi-head attention (softmax over the HEADS axis) on 8 trn2 NeuronCores.

Reference math (B=2, S=2048, D=512, H=8, Dk=64):
    q = split_heads(Q @ w_q.T + b_q)          # [B,H,S,Dk]
    scores = q @ k.T / sqrt(Dk)               # [B,H,Sq,Sk]
    probs = softmax(scores, axis=1)           # softmax over H (source quirk!)
    attn = probs @ v                          # [B,H,Sq,Dk]
    out = concat_heads(attn) @ w_o.T + b_o    # [B,S,D]

Softmax over H is local to each (b, sq, sk) position: sharding over
(batch x query rows) needs no cross-core communication.  Core c handles
batch c//4, query rows (c%4)*512 .. +512, with all 8 heads resident.

Design notes (learned from perfetto traces):
 - Input DMA is HBM-bound (~21 us for 7.5 MB/core) and rings share
   bandwidth proportionally to DESCRIPTOR SIZE, so kt/vt ship in a
   kb-major DRAM layout where every transfer moves 4KB-contiguous rows.
   Stages are striped over the three rings (sync/scalar HWDGE + gpsimd
   SWDGE) in dependency order: qt+wqt first, then wkt+K blocks, then
   wvt+V blocks, weights for the output projection last.
 - Steady-state loop is ACT-bound (4 EXPs/iter) with DVE a close second
   (head-sum tree + recip + normalize mul).  K blocks 2-3 and all V-tile
   projections are emitted inside the loop to shorten the serial
   prologue; V tiles project in pairs so the psum->SBUF copy (on ACT)
   costs one instruction per two tiles.
 - Normalize MULs for iterations 12-14 run on GPSIMD (pure SBUF op,
   3-iteration pipeline slack absorbs its erratic latency) so the
   end-of-loop DVE FIFO drains fast and the tail's critical chain is
   only tree(15) -> recip(15) -> mul(15a) before the last PV tile.
 - Tail: PV tiles 12+13 drain inside iteration 15; 14 right after; 15
   interleaves per head-pair with attnT copies and the c-outer output
   projection (psum accumulation order over chunks is free).  Output is
   written bf16 (error budget allows) and DMAd per qi-block.
 - Dummy matmuls bridge every PE-idle window (DMA startup, drain waits)
   to keep the HAM clock-gate at 8/8.
"""

import numpy as np

B, S, D, H, DK = 2, 2048, 512, 8, 64
NCORES = 8
CPB = NCORES // B          # cores per batch
QI = S // CPB              # query rows per core (512)
KJT = 128                  # kj tile (partition dim of scores)
NKJ = S // KJT             # 16 kj tiles
NC_, CH = 128, 4           # partitions, din chunks
KB = 4                     # 512-column blocks of kj
QS = S // KB               # 512
SCALE = 1.0 / np.sqrt(DK)  # folded into exp activation
LAG = 3                    # PV matmuls run LAG kj tiles behind the softmax


def _chunk(x, dt):
    """[512, F] -> [128, 4, F] with row = chunk*128 + p."""
    f = x.shape[1]
    return np.ascontiguousarray(
        np.ascontiguousarray(x).reshape(CH, NC_, f).transpose(1, 0, 2)
    ).astype(dt)


def _chunk_kb(x, dt):
    """[512, 2048] -> [128, kb, c, 512], row = c*128+p, col = kb*512+j.

    kb-major so each kb block is one DMA with 4KB-contiguous descriptors.
    """
    y = _chunk(x, np.float32)                      # [128, c, 2048]
    y = y.reshape(NC_, CH, KB, QS).transpose(0, 2, 1, 3)  # [128, kb, c, 512]
    return np.ascontiguousarray(y).astype(dt)


def _build(with_bias):
    from contextlib import ExitStack

    import concourse.bass as bass
    import concourse.mybir as mybir
    import concourse.tile as tile
    from concourse import bacc
    from concourse.dve_ops import (
        RECIP_APPROX_FAST_CONSTS as _RC,
        RECIPROCAL_APPROX_FAST as _RF,
    )

    fp32 = mybir.dt.float32
    bf16 = mybir.dt.bfloat16

    nc = bacc.Bacc(
        "TRN2",
        target_bir_lowering=False,
        debug=False,
        enable_asserts=False,
        num_devices=NCORES,
    )

    def din(name, shape):
        return nc.dram_tensor(name, shape, bf16, kind="ExternalInput").ap()

    qt_d = din("qt", [NC_, CH, QI])
    kt_d = din("kt", [NC_, KB, CH, QS])
    vt_d = din("vt", [NC_, KB, CH, QS])
    w_d = {n: din(n, [NC_, CH, D]) for n in ("wqt", "wkt", "wvt", "wot")}
    if with_bias:
        b_d = {n: din(n, [1, D]) for n in ("bq", "bk", "bv", "bo")}
    out_d = nc.dram_tensor("out", [QI, D], bf16, kind="ExternalOutput").ap()

    with tile.TileContext(nc) as tc, ExitStack() as ctx:
        acts = ctx.enter_context(tc.tile_pool(name="acts", bufs=1))
        sm = ctx.enter_context(tc.tile_pool(name="sm", bufs=2))
        pp = ctx.enter_context(tc.tile_pool(name="pp", bufs=LAG + 2))
        ps = ctx.enter_context(tc.tile_pool(name="ps", bufs=2, space="PSUM"))
        psa = ctx.enter_context(tc.tile_pool(name="psa", bufs=1, space="PSUM"))

        # ---- persistent SBUF tiles ----
        qTs = acts.tile([NC_, CH, QI], bf16, tag="qTs")
        kTsb = [
            acts.tile([NC_, CH, 1024], bf16, tag=f"kTs{b_}", name=f"kTs{b_}")
            for b_ in range(2)
        ]
        vs = acts.tile([NC_, NKJ, D], bf16, tag="vs", name="vs")
        attnT = acts.tile([NC_, CH, QI], bf16, tag="attnT")
        outsb = acts.tile([NC_, CH, D], bf16, tag="outsb")
        qraw = acts.tile([NC_, CH, QI], bf16, tag="qraw")
        kraw = acts.tile([NC_, KB, CH, QS], bf16, tag="kraw", name="kraw")
        vraw = acts.tile([NC_, KB, CH, QS], bf16, tag="vraw", name="vraw")
        wsb = {
            n: acts.tile([NC_, CH, D], bf16, tag=n, name=n)
            for n in ("wqt", "wkt", "wvt", "wot")
        }

        if with_bias:
            ones = acts.tile([1, 2 * D], bf16, tag="ones")
            nc.vector.memset(ones, 1.0)
            brow = {}
            for n in ("bq", "bk", "bv", "bo"):
                brow[n] = acts.tile([1, D], bf16, tag=n, name=n)
                nc.sync.dma_start(out=brow[n], in_=b_d[n])

        # ---- prologue DMAs: dependency-priority stages, 4KB descriptors ---
        # S1: Q proj inputs (+ wkt)
        nc.sync.dma_start(out=qraw, in_=qt_d)
        nc.scalar.dma_start(out=wsb["wqt"], in_=w_d["wqt"])
        nc.gpsimd.dma_start(out=wsb["wkt"], in_=w_d["wkt"])
        # S2: K blocks 0-1 (+ wvt)
        nc.sync.dma_start(out=kraw[:, 0], in_=kt_d[:, 0])
        nc.scalar.dma_start(out=kraw[:, 1], in_=kt_d[:, 1])
        nc.gpsimd.dma_start(out=wsb["wvt"], in_=w_d["wvt"])
        # S3: K blocks 2-3, V block 0
        nc.sync.dma_start(out=kraw[:, 2], in_=kt_d[:, 2])
        nc.scalar.dma_start(out=kraw[:, 3], in_=kt_d[:, 3])
        nc.gpsimd.dma_start(out=vraw[:, 0], in_=vt_d[:, 0])
        # S4: V blocks 1-2 (+ wot)
        nc.sync.dma_start(out=vraw[:, 1], in_=vt_d[:, 1])
        nc.scalar.dma_start(out=vraw[:, 2], in_=vt_d[:, 2])
        nc.gpsimd.dma_start(out=wsb["wot"], in_=w_d["wot"])
        # S5: V block 3
        nc.sync.dma_start(out=vraw[:, 3], in_=vt_d[:, 3])

        # dummy matmuls on garbage SBUF bridge PE-idle windows (DMA startup,
        # drain waits) so the HAM clock gate stays at 8/8.  Each call takes a
        # fresh psum tile from the rotating pool so it can never alias a
        # live score tile.
        _dn = [0]

        def dummy_mm(k):
            _dn[0] += 1
            wt = ps.tile([NC_, 2, 512], fp32, tag="sc", name=f"warm{_dn[0]}")
            for i in range(k):
                nc.tensor.matmul(
                    wt[:, i % 2, :],
                    lhsT=qTs[:, 0, 0:128],
                    rhs=qTs[:, 0, :],
                    start=True,
                    stop=True,
                )

        dummy_mm(12)

        def bias_mm(pt_ap, bname, col_slice):
            """rank-1 bias init: psum = bias-row (x) ones-row (or flipped)."""
            if col_slice is not None:  # bias along partitions
                lhsT = brow[bname][:, col_slice]
                rhs = ones[:, : pt_ap.shape[-1]]
            else:  # bias along free dim
                lhsT = ones[:, :128]
                rhs = brow[bname]
            nc.tensor.matmul(pt_ap, lhsT=lhsT, rhs=rhs, start=True, stop=False)

        # ---------------- prologue projections ----------------
        # Q: qT[dout, qi] = wqT[din,dout].T @ QT[din, qi]; two m per psum tile
        for mp in range(2):
            pt = ps.tile([NC_, 2, 512], fp32, tag="sc", name=f"qp{mp}")
            for j in range(2):
                m = 2 * mp + j
                if with_bias:
                    bias_mm(pt[:, j, :QI], "bq", slice(m * 128, (m + 1) * 128))
                for c in range(CH):
                    nc.tensor.matmul(
                        pt[:, j, :QI],
                        lhsT=wsb["wqt"][:, c, m * 128 : (m + 1) * 128],
                        rhs=qraw[:, c, :],
                        start=(c == 0 and not with_bias),
                        stop=(c == CH - 1),
                    )
            if mp == 0:
                nc.scalar.copy(qTs[:, 0:2, :], pt[:, :, :QI])
            else:
                nc.vector.tensor_copy(qTs[:, 2:4, :], pt[:, :, :QI])

        def emit_kproj(kb, mp):
            """one dout m-pair of K proj for 512-col block kb + kTs copy."""
            pt = ps.tile([NC_, 2, 512], fp32, tag="sc", name=f"kp{kb}_{mp}")
            for j in range(2):
                m = 2 * mp + j
                if with_bias:
                    bias_mm(pt[:, j, :], "bk", slice(m * 128, (m + 1) * 128))
                for c in range(CH):
                    nc.tensor.matmul(
                        pt[:, j, :],
                        lhsT=wsb["wkt"][:, c, m * 128 : (m + 1) * 128],
                        rhs=kraw[:, kb, c, :],
                        start=(c == 0 and not with_bias),
                        stop=(c == CH - 1),
                    )
            dst = kTsb[kb // 2][
                :, 2 * mp : 2 * mp + 2, (kb % 2) * 512 : (kb % 2 + 1) * 512
            ]
            if mp == 0:
                nc.scalar.copy(dst, pt)
            else:
                nc.vector.tensor_copy(dst, pt)

        # bridge the kb0 DMA-arrival gap after Q proj
        dummy_mm(4)
        for kb in range(KB):  # all K blocks in prologue (in-loop projection
            for mp in range(2):  # serializes on the 2-slot psum rotation)
                emit_kproj(kb, mp)

        def emit_vpair(t0, engine):
            """project v tiles t0, t0+1 into one psum tile + one copy."""
            kvt = ps.tile([NC_, 2, 512], fp32, tag="sc", name=f"vp{t0}")
            for j in range(2):
                vt_i = t0 + j
                if with_bias:
                    bias_mm(kvt[:, j, :], "bv", None)
                for c in range(CH):
                    nc.tensor.matmul(
                        kvt[:, j, :],
                        lhsT=vraw[
                            :, vt_i // 4, c, (vt_i % 4) * 128 : (vt_i % 4 + 1) * 128
                        ],
                        rhs=wsb["wvt"][:, c, :],
                        start=(c == 0 and not with_bias),
                        stop=(c == CH - 1),
                    )
            engine(vs[:, t0 : t0 + 2, :], kvt)

        emit_vpair(0, nc.vector.tensor_copy)
        emit_vpair(2, nc.scalar.copy)

        # ---------------- fused attention + pipelined K/V projection ------
        at = [
            psa.tile([NC_, 512], fp32, tag=f"at{i}", name=f"at{i}")
            for i in range(4)
        ]

        def emit_pv(td, pr, m):
            """PV matmuls for heads 2m, 2m+1 of kj tile td."""
            for h in (2 * m, 2 * m + 1):
                po = (h % 2) * 64
                nc.tensor.matmul(
                    at[h // 2][po : po + 64, :QI],
                    lhsT=vs[:, td, h * 64 : (h + 1) * 64],
                    rhs=pr[:, h, :],
                    start=(td == 0),
                    stop=(td == NKJ - 1),
                )

        pending = []
        for t in range(NKJ):
            exp_t = sm.tile([NC_, H, QI], bf16, tag="exp", bufs=3)
            pv = pending.pop(0) if len(pending) >= LAG else None
            do_kv = t < NKJ - CH
            last = t == NKJ - 1
            # last iteration has no V proj: drain a second pending PV tile
            pv2 = pending.pop(0) if last else None
            if last:
                sA = sm.tile([NC_, 2, QI], bf16, tag="sA")

            for m in range(4):
                spt = ps.tile([NC_, 2, 512], fp32, tag="sc", name=f"s{t}_{m}")
                for j in range(2):
                    po = j * 64
                    nc.tensor.matmul(
                        spt[:, j, :QI],
                        lhsT=kTsb[t // 8][
                            po : po + 64, m, (t % 8) * 128 : (t % 8 + 1) * 128
                        ],
                        rhs=qTs[po : po + 64, m, :],
                        start=True,
                        stop=True,
                    )
                if pv is not None:
                    emit_pv(pv[0], pv[1], m)
                if pv2 is not None:
                    emit_pv(pv2[0], pv2[1], m)
                nc.scalar.activation(
                    exp_t[:, 2 * m : 2 * m + 2, :],
                    spt[:, :, :],
                    mybir.ActivationFunctionType.Exp,
                    scale=SCALE,
                )
                if m == 1:
                    if last:
                        # latency-optimized tree for the final tile
                        nc.vector.tensor_add(
                            sA, exp_t[:, 0:2, :], exp_t[:, 2:4, :]
                        )
                    if t < LAG:
                        # pre-PV iterations are PE-sparse: cheap warm-keeper
                        # (a dummy slot frees after its own MMs, ~0.4 us,
                        # unlike projection slots which wait on ACT copies)
                        dummy_mm(2)
                    if do_kv:
                        # single V tile per iteration: a [128,1,512] psum
                        # tile keeps the 2-slot rotation fluid
                        vt_i = t + CH
                        kvt = ps.tile(
                            [NC_, 1, 512], fp32, tag="sc", name=f"kv{t}"
                        )
                        if with_bias:
                            bias_mm(kvt[:, 0, :], "bv", None)
                        for c in range(CH):
                            nc.tensor.matmul(
                                kvt[:, 0, :],
                                lhsT=vraw[
                                    :,
                                    vt_i // 4,
                                    c,
                                    (vt_i % 4) * 128 : (vt_i % 4 + 1) * 128,
                                ],
                                rhs=wsb["wvt"][:, c, :],
                                start=(c == 0 and not with_bias),
                                stop=(c == CH - 1),
                            )
                        nc.scalar.copy(vs[:, vt_i, :], kvt[:, 0, :])

            # head-sum tree, all on DVE
            ssum = sm.tile([NC_, QI], bf16, tag="ssum")
            if last:
                sB = sm.tile([NC_, 2, QI], bf16, tag="sB")
                nc.vector.tensor_add(sB, exp_t[:, 4:6, :], exp_t[:, 6:8, :])
                sAB = sm.tile([NC_, 2, QI], bf16, tag="s2")
                nc.vector.tensor_add(sAB, sA, sB)
                nc.vector.tensor_add(ssum, sAB[:, 0, :], sAB[:, 1, :])
            else:
                s4 = sm.tile([NC_, 4, QI], bf16, tag="s4")
                nc.vector.tensor_add(s4, exp_t[:, 0:4, :], exp_t[:, 4:8, :])
                s2 = sm.tile([NC_, 2, QI], bf16, tag="s2")
                nc.vector.tensor_add(s2, s4[:, 0:2, :], s4[:, 2:4, :])
                nc.vector.tensor_add(ssum, s2[:, 0, :], s2[:, 1, :])
            # fast reciprocal; bf16 in/out
            r = sm.tile([NC_, QI], bf16, tag="r")
            nc.vector._custom_dve(
                _RF, out=r, in0=ssum, s0=_RC["s0"], s1=_RC["s1"], imm2=_RC["imm2"]
            )

            # normalize: r broadcast over heads.  The last two iterations
            # split the mul so the tail's PV matmuls can start after the
            # first half.
            pr = pp.tile([NC_, H, QI], bf16, tag="probs", name=f"pr{t}")
            if t >= NKJ - 2:
                nc.vector.tensor_mul(
                    pr[:, 0:4, :],
                    exp_t[:, 0:4, :],
                    r[:, None, :].broadcast_to([NC_, 4, QI]),
                )
                nc.vector.tensor_mul(
                    pr[:, 4:8, :],
                    exp_t[:, 4:8, :],
                    r[:, None, :].broadcast_to([NC_, 4, QI]),
                )
            else:
                nc.vector.tensor_mul(
                    pr,
                    exp_t,
                    r[:, None, :].broadcast_to([NC_, H, QI]),
                )
            pending.append((t, pr))

        # ---------------- tail: drain + output projection, interleaved ----
        (td14, pr14), (td15, pr15) = pending
        dummy_mm(6)
        for m in range(4):
            emit_pv(td14, pr14, m)

        otq = [
            ps.tile([NC_, 2, 512], fp32, tag="sc", name=f"oq{i}")
            for i in range(2)
        ]
        if with_bias:
            for qb in range(4):
                bias_mm(otq[qb // 2][:, qb % 2, :], "bo", None)
        for m in range(4):
            emit_pv(td15, pr15, m)
            if m % 2 == 0:
                nc.vector.tensor_copy(attnT[:, m, :], at[m][:, :QI])
            else:
                nc.scalar.copy(attnT[:, m, :], at[m][:, :QI])
            for qb in range(4):
                nc.tensor.matmul(
                    otq[qb // 2][:, qb % 2, :],
                    lhsT=attnT[:, m, qb * 128 : (qb + 1) * 128],
                    rhs=wsb["wot"][:, m, :],
                    start=(m == 0 and not with_bias),
                    stop=(m == CH - 1),
                )
        out_re = out_d.rearrange("(m p) o -> p m o", p=NC_)
        for qb in range(4):
            if qb % 2 == 0:
                nc.scalar.copy(outsb[:, qb, :], otq[qb // 2][:, qb % 2, :])
            else:
                nc.vector.tensor_copy(outsb[:, qb, :], otq[qb // 2][:, qb % 2, :])
            (nc.sync if qb % 2 == 0 else nc.scalar).dma_start(
                out=out_re[:, qb, :], in_=outsb[:, qb, :]
            )

    nc.compile()
    return nc


_CACHE = {}


def kernel(Q, K, V, w_q, b_q, w_k, b_k, w_v, b_v, w_o, b_o, _trace=False):
    import ml_dtypes
    from concourse import bass_utils

    bf = ml_dtypes.bfloat16
    Q = np.asarray(Q, np.float32)
    K = np.asarray(K, np.float32)
    V = np.asarray(V, np.float32)
    with_bias = any(
        np.any(np.asarray(b) != 0) for b in (b_q, b_k, b_v, b_o)
    )

    if ("nc", with_bias) not in _CACHE:
        _CACHE[("nc", with_bias)] = _build(with_bias)
    nc = _CACHE[("nc", with_bias)]

    wmaps = {
        "wqt": _chunk(np.asarray(w_q, np.float32).T, bf),
        "wkt": _chunk(np.asarray(w_k, np.float32).T, bf),
        "wvt": _chunk(np.asarray(w_v, np.float32).T, bf),
        "wot": _chunk(np.asarray(w_o, np.float32).T, bf),
    }
    if with_bias:
        for n, b in (("bq", b_q), ("bk", b_k), ("bv", b_v), ("bo", b_o)):
            wmaps[n] = np.ascontiguousarray(
                np.asarray(b, np.float32).reshape(1, D)
            ).astype(bf)

    in_maps = []
    for c in range(NCORES):
        b = c // CPB
        s0 = (c % CPB) * QI
        in_maps.append(
            dict(
                wmaps,
                qt=_chunk(Q[b, s0 : s0 + QI, :].T, bf),
                kt=_chunk_kb(K[b].T, bf),
                vt=_chunk_kb(V[b].T, bf),
            )
        )

    res = bass_utils.run_bass_kernel_spmd(
        nc, in_maps, core_ids=list(range(NCORES)), trace=_trace
    )

    out = np.empty((B, S, D), np.float32)
    for c in range(NCORES):
        b = c // CPB
        s0 = (c % CPB) * QI
        out[b, s0 : s0 + QI, :] = np.asarray(res.results[c]["out"]).astype(
            np.float32
        )
    if _trace:
        kernel._last_results = res
    return out


# revision 36
# speedup vs baseline: 1.2373x; 1.0146x over previous
"""Multi-head attention (softmax over the HEADS axis) on 8 trn2 NeuronCores.

Reference math (B=2, S=2048, D=512, H=8, Dk=64):
    q = split_heads(Q @ w_q.T + b_q)          # [B,H,S,Dk]
    scores = q @ k.T / sqrt(Dk)               # [B,H,Sq,Sk]
    probs = softmax(scores, axis=1)           # softmax over H (source quirk!)
    attn = probs @ v                          # [B,H,Sq,Dk]
    out = concat_heads(attn) @ w_o.T + b_o    # [B,S,D]

Softmax over H is local to each (b, sq, sk) position: sharding over
(batch x query rows) needs no cross-core communication.  Core c handles
batch c//4, query rows (c%4)*512 .. +512, with all 8 heads resident.

Design notes (learned from perfetto traces):
 - Input DMA is HBM-bound (~21 us for 7.5 MB/core) and rings share
   bandwidth proportionally to DESCRIPTOR SIZE, so kt/vt ship in a
   kb-major DRAM layout where every transfer moves 4KB-contiguous rows.
   Stages are striped over the three rings (sync/scalar HWDGE + gpsimd
   SWDGE) in dependency order: qt+wqt first, then wkt+K blocks, then
   wvt+V blocks, weights for the output projection last.
 - Steady-state loop (~4.9 us/iter) is ACT-paced (4 EXPs + V-copy) with
   DVE the throughput bound (head-sum tree + recip + normalize mul,
   ~5.3 us busy/iter at its measured ~0.6 ns/elem) whose backlog drains
   after the last EXP.  All Q/K/V-0..3 projections stay in the prologue:
   in-loop projection blocks serialize on the 2-slot psum rotation
   because their psum->SBUF copy queues behind the EXPs (measured 3x).
   Only the single-V-tile-per-iteration pattern (1-bank psum tile,
   ~0.7 us copy) fits in the loop's slack.
 - GPSIMD is unusable for elementwise work (tensor_mul measured 7.9-8.7
   us for [128,8,512], 2.4x the model, serialized) and has no PSUM port.
 - Tail: PV tiles 12+13 drain inside iteration 15 (double-pop); 14 right
   after; 15 interleaves per head-pair with attnT copies and the c-outer
   output projection (psum accumulation order over chunks is free); the
   muls of iterations 14/15 are split in halves so PV can start early.
   Output is written bf16 (error budget allows) and DMAd per qi-block.
 - Dummy matmuls (always into a freshly allocated pool tile, never a
   stale handle) bridge PE-idle windows to keep the HAM clock at 8/8.
"""

import numpy as np

B, S, D, H, DK = 2, 2048, 512, 8, 64
NCORES = 8
CPB = NCORES // B          # cores per batch
QI = S // CPB              # query rows per core (512)
KJT = 128                  # kj tile (partition dim of scores)
NKJ = S // KJT             # 16 kj tiles
NC_, CH = 128, 4           # partitions, din chunks
KB = 4                     # 512-column blocks of kj
QS = S // KB               # 512
SCALE = 1.0 / np.sqrt(DK)  # folded into exp activation
LAG = 3                    # PV matmuls run LAG kj tiles behind the softmax


def _chunk(x, dt):
    """[512, F] -> [128, 4, F] with row = chunk*128 + p."""
    f = x.shape[1]
    return np.ascontiguousarray(
        np.ascontiguousarray(x).reshape(CH, NC_, f).transpose(1, 0, 2)
    ).astype(dt)


def _chunk_kb(x, dt):
    """[512, 2048] -> [128, kb, c, 512], row = c*128+p, col = kb*512+j.

    kb-major so each kb block is one DMA with 4KB-contiguous descriptors.
    """
    y = _chunk(x, np.float32)                      # [128, c, 2048]
    y = y.reshape(NC_, CH, KB, QS).transpose(0, 2, 1, 3)  # [128, kb, c, 512]
    return np.ascontiguousarray(y).astype(dt)


def _build(with_bias):
    from contextlib import ExitStack

    import concourse.bass as bass
    import concourse.mybir as mybir
    import concourse.tile as tile
    from concourse import bacc
    from concourse.dve_ops import (
        RECIP_APPROX_FAST_CONSTS as _RC,
        RECIPROCAL_APPROX_FAST as _RF,
    )

    fp32 = mybir.dt.float32
    bf16 = mybir.dt.bfloat16

    nc = bacc.Bacc(
        "TRN2",
        target_bir_lowering=False,
        debug=False,
        enable_asserts=False,
        num_devices=NCORES,
    )

    def din(name, shape):
        return nc.dram_tensor(name, shape, bf16, kind="ExternalInput").ap()

    qt_d = din("qt", [NC_, CH, QI])
    kt_d = din("kt", [NC_, KB, CH, QS])
    vt_d = din("vt", [NC_, KB, CH, QS])
    w_d = {n: din(n, [NC_, CH, D]) for n in ("wqt", "wkt", "wvt", "wot")}
    if with_bias:
        b_d = {n: din(n, [1, D]) for n in ("bq", "bk", "bv", "bo")}
    out_d = nc.dram_tensor("out", [QI, D], bf16, kind="ExternalOutput").ap()

    with tile.TileContext(nc) as tc, ExitStack() as ctx:
        acts = ctx.enter_context(tc.tile_pool(name="acts", bufs=1))
        sm = ctx.enter_context(tc.tile_pool(name="sm", bufs=2))
        pp = ctx.enter_context(tc.tile_pool(name="pp", bufs=LAG + 2))
        ps = ctx.enter_context(tc.tile_pool(name="ps", bufs=2, space="PSUM"))
        psa = ctx.enter_context(tc.tile_pool(name="psa", bufs=1, space="PSUM"))

        # ---- persistent SBUF tiles ----
        qTs = acts.tile([NC_, CH, QI], bf16, tag="qTs")
        kTsb = [
            acts.tile([NC_, CH, 1024], bf16, tag=f"kTs{b_}", name=f"kTs{b_}")
            for b_ in range(2)
        ]
        vs = acts.tile([NC_, NKJ, D], bf16, tag="vs", name="vs")
        attnT = acts.tile([NC_, CH, QI], bf16, tag="attnT")
        outsb = acts.tile([NC_, CH, D], bf16, tag="outsb")
        qraw = acts.tile([NC_, CH, QI], bf16, tag="qraw")
        kraw = acts.tile([NC_, KB, CH, QS], bf16, tag="kraw", name="kraw")
        vraw = acts.tile([NC_, KB, CH, QS], bf16, tag="vraw", name="vraw")
        wsb = {
            n: acts.tile([NC_, CH, D], bf16, tag=n, name=n)
            for n in ("wqt", "wkt", "wvt", "wot")
        }

        if with_bias:
            ones = acts.tile([1, 2 * D], bf16, tag="ones")
            nc.vector.memset(ones, 1.0)
            brow = {}
            for n in ("bq", "bk", "bv", "bo"):
                brow[n] = acts.tile([1, D], bf16, tag=n, name=n)
                nc.sync.dma_start(out=brow[n], in_=b_d[n])

        # ---- prologue DMAs: dependency-priority stages, 4KB descriptors ---
        # S1: Q proj inputs (+ wkt)
        nc.sync.dma_start(out=qraw, in_=qt_d)
        nc.scalar.dma_start(out=wsb["wqt"], in_=w_d["wqt"])
        nc.gpsimd.dma_start(out=wsb["wkt"], in_=w_d["wkt"])
        # S2: K blocks 0-1 (+ wvt)
        nc.sync.dma_start(out=kraw[:, 0], in_=kt_d[:, 0])
        nc.scalar.dma_start(out=kraw[:, 1], in_=kt_d[:, 1])
        nc.gpsimd.dma_start(out=wsb["wvt"], in_=w_d["wvt"])
        # S3: K blocks 2-3, V block 0
        nc.sync.dma_start(out=kraw[:, 2], in_=kt_d[:, 2])
        nc.scalar.dma_start(out=kraw[:, 3], in_=kt_d[:, 3])
        nc.gpsimd.dma_start(out=vraw[:, 0], in_=vt_d[:, 0])
        # S4: V blocks 1-2 (+ wot)
        nc.sync.dma_start(out=vraw[:, 1], in_=vt_d[:, 1])
        nc.scalar.dma_start(out=vraw[:, 2], in_=vt_d[:, 2])
        nc.gpsimd.dma_start(out=wsb["wot"], in_=w_d["wot"])
        # S5: V block 3
        nc.sync.dma_start(out=vraw[:, 3], in_=vt_d[:, 3])

        # dummy matmuls on garbage SBUF bridge PE-idle windows (DMA startup,
        # drain waits) so the HAM clock gate stays at 8/8.  Each call takes a
        # fresh psum tile from the rotating pool so it can never alias a
        # live score tile.
        _dn = [0]

        def dummy_mm(k):
            _dn[0] += 1
            wt = ps.tile([NC_, 2, 512], fp32, tag="sc", name=f"warm{_dn[0]}")
            for i in range(k):
                nc.tensor.matmul(
                    wt[:, i % 2, :],
                    lhsT=qTs[:, 0, 0:128],
                    rhs=qTs[:, 0, :],
                    start=True,
                    stop=True,
                )

        dummy_mm(12)

        def bias_mm(pt_ap, bname, col_slice):
            """rank-1 bias init: psum = bias-row (x) ones-row (or flipped)."""
            if col_slice is not None:  # bias along partitions
                lhsT = brow[bname][:, col_slice]
                rhs = ones[:, : pt_ap.shape[-1]]
            else:  # bias along free dim
                lhsT = ones[:, :128]
                rhs = brow[bname]
            nc.tensor.matmul(pt_ap, lhsT=lhsT, rhs=rhs, start=True, stop=False)

        # ---------------- prologue projections ----------------
        # Q: qT[dout, qi] = wqT[din,dout].T @ QT[din, qi]; two m per psum tile
        for mp in range(2):
            pt = ps.tile([NC_, 2, 512], fp32, tag="sc", name=f"qp{mp}")
            for j in range(2):
                m = 2 * mp + j
                if with_bias:
                    bias_mm(pt[:, j, :QI], "bq", slice(m * 128, (m + 1) * 128))
                for c in range(CH):
                    nc.tensor.matmul(
                        pt[:, j, :QI],
                        lhsT=wsb["wqt"][:, c, m * 128 : (m + 1) * 128],
                        rhs=qraw[:, c, :],
                        start=(c == 0 and not with_bias),
                        stop=(c == CH - 1),
                    )
            if mp == 0:
                nc.scalar.copy(qTs[:, 0:2, :], pt[:, :, :QI])
            else:
                nc.vector.tensor_copy(qTs[:, 2:4, :], pt[:, :, :QI])

        def emit_kproj(kb, mp):
            """one dout m-pair of K proj for 512-col block kb + kTs copy."""
            pt = ps.tile([NC_, 2, 512], fp32, tag="sc", name=f"kp{kb}_{mp}")
            for j in range(2):
                m = 2 * mp + j
                if with_bias:
                    bias_mm(pt[:, j, :], "bk", slice(m * 128, (m + 1) * 128))
                for c in range(CH):
                    nc.tensor.matmul(
                        pt[:, j, :],
                        lhsT=wsb["wkt"][:, c, m * 128 : (m + 1) * 128],
                        rhs=kraw[:, kb, c, :],
                        start=(c == 0 and not with_bias),
                        stop=(c == CH - 1),
                    )
            dst = kTsb[kb // 2][
                :, 2 * mp : 2 * mp + 2, (kb % 2) * 512 : (kb % 2 + 1) * 512
            ]
            if mp == 0:
                nc.scalar.copy(dst, pt)
            else:
                nc.vector.tensor_copy(dst, pt)

        for kb in range(KB):  # all K blocks in prologue (in-loop projection
            for mp in range(2):  # serializes on the 2-slot psum rotation)
                emit_kproj(kb, mp)

        def emit_vpair(t0, engine):
            """project v tiles t0, t0+1 into one psum tile + one copy."""
            kvt = ps.tile([NC_, 2, 512], fp32, tag="sc", name=f"vp{t0}")
            for j in range(2):
                vt_i = t0 + j
                if with_bias:
                    bias_mm(kvt[:, j, :], "bv", None)
                for c in range(CH):
                    nc.tensor.matmul(
                        kvt[:, j, :],
                        lhsT=vraw[
                            :, vt_i // 4, c, (vt_i % 4) * 128 : (vt_i % 4 + 1) * 128
                        ],
                        rhs=wsb["wvt"][:, c, :],
                        start=(c == 0 and not with_bias),
                        stop=(c == CH - 1),
                    )
            engine(vs[:, t0 : t0 + 2, :], kvt)

        emit_vpair(0, nc.vector.tensor_copy)
        emit_vpair(2, nc.scalar.copy)

        # ---------------- fused attention + pipelined K/V projection ------
        at = [
            psa.tile([NC_, 512], fp32, tag=f"at{i}", name=f"at{i}")
            for i in range(4)
        ]

        def emit_pv(td, pr, m):
            """PV matmuls for heads 2m, 2m+1 of kj tile td."""
            for h in (2 * m, 2 * m + 1):
                po = (h % 2) * 64
                nc.tensor.matmul(
                    at[h // 2][po : po + 64, :QI],
                    lhsT=vs[:, td, h * 64 : (h + 1) * 64],
                    rhs=pr[:, h, :],
                    start=(td == 0),
                    stop=(td == NKJ - 1),
                )

        pending = []
        for t in range(NKJ):
            exp_t = sm.tile([NC_, H, QI], bf16, tag="exp", bufs=3)
            pv = pending.pop(0) if len(pending) >= LAG else None
            do_kv = t < NKJ - CH
            last = t == NKJ - 1
            # last iteration has no V proj: drain a second pending PV tile
            pv2 = pending.pop(0) if last else None
            if last:
                sA = sm.tile([NC_, 2, QI], bf16, tag="sA")

            for m in range(4):
                spt = ps.tile([NC_, 2, 512], fp32, tag="sc", name=f"s{t}_{m}")
                for j in range(2):
                    po = j * 64
                    nc.tensor.matmul(
                        spt[:, j, :QI],
                        lhsT=kTsb[t // 8][
                            po : po + 64, m, (t % 8) * 128 : (t % 8 + 1) * 128
                        ],
                        rhs=qTs[po : po + 64, m, :],
                        start=True,
                        stop=True,
                    )
                if pv is not None:
                    emit_pv(pv[0], pv[1], m)
                if pv2 is not None:
                    emit_pv(pv2[0], pv2[1], m)
                nc.scalar.activation(
                    exp_t[:, 2 * m : 2 * m + 2, :],
                    spt[:, :, :],
                    mybir.ActivationFunctionType.Exp,
                    scale=SCALE,
                )
                if m == 1:
                    if last:
                        # latency-optimized tree for the final tile
                        nc.vector.tensor_add(
                            sA, exp_t[:, 0:2, :], exp_t[:, 2:4, :]
                        )
                    if do_kv:
                        # single V tile per iteration: a [128,1,512] psum
                        # tile keeps the 2-slot rotation fluid
                        vt_i = t + CH
                        kvt = ps.tile(
                            [NC_, 1, 512], fp32, tag="sc", name=f"kv{t}"
                        )
                        if with_bias:
                            bias_mm(kvt[:, 0, :], "bv", None)
                        for c in range(CH):
                            nc.tensor.matmul(
                                kvt[:, 0, :],
                                lhsT=vraw[
                                    :,
                                    vt_i // 4,
                                    c,
                                    (vt_i % 4) * 128 : (vt_i % 4 + 1) * 128,
                                ],
                                rhs=wsb["wvt"][:, c, :],
                                start=(c == 0 and not with_bias),
                                stop=(c == CH - 1),
                            )
                        nc.scalar.copy(vs[:, vt_i, :], kvt[:, 0, :])

            # head-sum tree, all on DVE
            ssum = sm.tile([NC_, QI], bf16, tag="ssum")
            if last:
                sB = sm.tile([NC_, 2, QI], bf16, tag="sB")
                nc.vector.tensor_add(sB, exp_t[:, 4:6, :], exp_t[:, 6:8, :])
                sAB = sm.tile([NC_, 2, QI], bf16, tag="s2")
                nc.vector.tensor_add(sAB, sA, sB)
                nc.vector.tensor_add(ssum, sAB[:, 0, :], sAB[:, 1, :])
            else:
                s4 = sm.tile([NC_, 4, QI], bf16, tag="s4")
                nc.vector.tensor_add(s4, exp_t[:, 0:4, :], exp_t[:, 4:8, :])
                s2 = sm.tile([NC_, 2, QI], bf16, tag="s2")
                nc.vector.tensor_add(s2, s4[:, 0:2, :], s4[:, 2:4, :])
                nc.vector.tensor_add(ssum, s2[:, 0, :], s2[:, 1, :])
            # fast reciprocal; bf16 in/out
            r = sm.tile([NC_, QI], bf16, tag="r")
            nc.vector._custom_dve(
                _RF, out=r, in0=ssum, s0=_RC["s0"], s1=_RC["s1"], imm2=_RC["imm2"]
            )

            # normalize: r broadcast over heads.  The last two iterations
            # split the mul so the tail's PV matmuls can start after the
            # first half.
            pr = pp.tile([NC_, H, QI], bf16, tag="probs", name=f"pr{t}")
            if t >= NKJ - 2:
                nc.vector.tensor_mul(
                    pr[:, 0:4, :],
                    exp_t[:, 0:4, :],
                    r[:, None, :].broadcast_to([NC_, 4, QI]),
                )
                nc.vector.tensor_mul(
                    pr[:, 4:8, :],
                    exp_t[:, 4:8, :],
                    r[:, None, :].broadcast_to([NC_, 4, QI]),
                )
            else:
                nc.vector.tensor_mul(
                    pr,
                    exp_t,
                    r[:, None, :].broadcast_to([NC_, H, QI]),
                )
            pending.append((t, pr))

        # ---------------- tail: drain + output projection, interleaved ----
        (td14, pr14), (td15, pr15) = pending
        dummy_mm(6)
        for m in range(4):
            emit_pv(td14, pr14, m)

        otq = [
            ps.tile([NC_, 2, 512], fp32, tag="sc", name=f"oq{i}")
            for i in range(2)
        ]
        if with_bias:
            for qb in range(4):
                bias_mm(otq[qb // 2][:, qb % 2, :], "bo", None)
        for m in range(4):
            emit_pv(td15, pr15, m)
            if m % 2 == 0:
                nc.vector.tensor_copy(attnT[:, m, :], at[m][:, :QI])
            else:
                nc.scalar.copy(attnT[:, m, :], at[m][:, :QI])
            for qb in range(4):
                nc.tensor.matmul(
                    otq[qb // 2][:, qb % 2, :],
                    lhsT=attnT[:, m, qb * 128 : (qb + 1) * 128],
                    rhs=wsb["wot"][:, m, :],
                    start=(m == 0 and not with_bias),
                    stop=(m == CH - 1),
                )
        out_re = out_d.rearrange("(m p) o -> p m o", p=NC_)
        for qb in range(4):
            if qb % 2 == 0:
                nc.scalar.copy(outsb[:, qb, :], otq[qb // 2][:, qb % 2, :])
            else:
                nc.vector.tensor_copy(outsb[:, qb, :], otq[qb // 2][:, qb % 2, :])
            (nc.sync if qb % 2 == 0 else nc.scalar).dma_start(
                out=out_re[:, qb, :], in_=outsb[:, qb, :]
            )

    nc.compile()
    return nc


_CACHE = {}


def kernel(Q, K, V, w_q, b_q, w_k, b_k, w_v, b_v, w_o, b_o, _trace=False):
    import ml_dtypes
    from concourse import bass_utils

    bf = ml_dtypes.bfloat16
    Q = np.asarray(Q, np.float32)
    K = np.asarray(K, np.float32)
    V = np.asarray(V, np.float32)
    with_bias = any(
        np.any(np.asarray(b) != 0) for b in (b_q, b_k, b_v, b_o)
    )

    if ("nc", with_bias) not in _CACHE:
        _CACHE[("nc", with_bias)] = _build(with_bias)
    nc = _CACHE[("nc", with_bias)]

    wmaps = {
        "wqt": _chunk(np.asarray(w_q, np.float32).T, bf),
        "wkt": _chunk(np.asarray(w_k, np.float32).T, bf),
        "wvt": _chunk(np.asarray(w_v, np.float32).T, bf),
        "wot": _chunk(np.asarray(w_o, np.float32).T, bf),
    }
    if with_bias:
        for n, b in (("bq", b_q), ("bk", b_k), ("bv", b_v), ("bo", b_o)):
            wmaps[n] = np.ascontiguousarray(
                np.asarray(b, np.float32).reshape(1, D)
            ).astype(bf)

    in_maps = []
    for c in range(NCORES):
        b = c // CPB
        s0 = (c % CPB) * QI
        in_maps.append(
            dict(
                wmaps,
                qt=_chunk(Q[b, s0 : s0 + QI, :].T, bf),
                kt=_chunk_kb(K[b].T, bf),
                vt=_chunk_kb(V[b].T, bf),
            )
        )

    res = bass_utils.run_bass_kernel_spmd(
        nc, in_maps, core_ids=list(range(NCORES)), trace=_trace
    )

    out = np.empty((B, S, D), np.float32)
    for c in range(NCORES):
        b = c // CPB
        s0 = (c % CPB) * QI
        out[b, s0 : s0 + QI, :] = np.asarray(res.results[c]["out"]).astype(
            np.float32
        )
    if _trace:
        kernel._last_results = res
    return out
